# revision 13
# baseline (speedup 1.0000x reference)
"""Trainium2 Bass kernel for nn_CP_Based (CP-decomposition interaction layer).

Math (full problem):
    t[b,f,r,u] = sum_d X[b,f,d] * K[d,r,f,u]      (B=1024, F=64, D=4, R=32, U=128)
    had[b,r,u] = prod_f t[b,f,r,u]
    out[b,u]   = sum_r had[b,r,u]

Strategy (v2):
  * Shard (units x batch) = 4 x 2 over 8 cores: each core gets 512 batch
    rows (4 partition tiles) and 32 units (RU_loc = 32r*32u = 1024).
  * Feature grouping (host-side weight repack): merge g features into one
    "factor" via outer products of the per-feature slices; one K=4^g
    matmul computes the product of g per-feature terms. Mix of quads
    (g=4, K=256 -> 2 accumulating K=128 passes) and triples (g=3, K=64,
    two factors row-packed per PE pass via tile_position).
  * fp16 matmul inputs (fp32 PSUM accumulate): 1 cycle/row on PE (vs 4
    for fp32) and half the DMA bytes. Verified rel err ~8e-3 < 2e-2.
  * The hadamard over factors is elementwise and PSUM-exit-bound: PSUM
    can only be read by DVE and Act (1 elem/cycle/lane each; GpSimd and
    DMA have no PSUM port). Pipeline:
      - DVE chain: running product P *= ps directly from PSUM.
      - Act (ScalarE): copies other factor tiles PSUM->SBUF fp32.
      - Merges of copied tiles into a second running product SA via
        scalar_tensor_tensor (STT supports the 2x_2p DVE perf mode:
        0.5 cyc/elem fp32 all-SBUF), split between DVE and GpSimd.
      - Final: F = P*SA (STT), strided tensor_reduce over r, DMA out.

kernel() takes FULL inputs, repacks on host, runs SPMD on 8 cores,
reassembles the full [1024, 128] output.
"""

import numpy as np

B, F, D, R, U = 1024, 64, 4, 32, 128
NCORES = 8
UB_SHARD = 4                  # units split
BB_SHARD = 2                  # batch split
BLOC = B // BB_SHARD          # 512 batch rows per core
NBT = BLOC // 128             # 4 partition tiles per core
ULOC = U // UB_SHARD          # 32 units per core
RULOC = R * ULOC              # 1024

# Factor mix: NQ quads (4 feats) + NT triples (3 feats); 4*NQ + 3*NT = 64
NQ = 4
NT = 16
assert 4 * NQ + 3 * NT == F and NT % 2 == 0

# Per-factor-tile exit routing per batch-tile, in production order:
# 'v' -> DVE chain (P *= ps, fp32, ~1.4us/tile measured).
# 'a' -> Act copy PSUM->SBUF as fp16 (~1.25us; single t3 tiles are fp16
#        range-safe, unlike products). Pairs of copies merge on DVE via
#        fp16 tensor_tensor (2x_1p mode is real on HW: ~0.85us) into
#        fp16 pair-products PPH (range-safe to ~1e4 < 65504). The PPH
#        chain then runs in fp32 mostly on GpSimd (SBUF only, ~2.8us),
#        with the leftovers merged into P on DVE.
# fp32 gets no DVE perf mode on this silicon (STT 2x_2p is cost-model
# fiction), so balance is v*1.4 vs a*1.25 on Act + 0.85/pair on DVE.
NFAC = NQ + NT                # 20
ROUTE = ['v', 'a', 'a', 'a', 'a', 'v', 'a', 'a', 'a', 'a', 'v', 'a',
         'a', 'a', 'a', 'v', 'a', 'a', 'a', 'a']
assert len(ROUTE) == NFAC and ROUTE.count('a') % 2 == 0
N_ACT = ROUTE.count('a')      # 16 -> 8 fp16 pair-products per bt
NPP = N_ACT // 2
# pair-products j=1..N_POOL_PP chain on Pool into SA; later ones fold
# into P on DVE (keeps Pool under its ~2.4us/op budget).
N_POOL_PP = 6

_cached = {}


def _factor_feats():
    """Feature index tuples for each factor: NQ quads then NT triples."""
    feats = []
    f = 0
    for _ in range(NQ):
        feats.append(tuple(range(f, f + 4)))
        f += 4
    for _ in range(NT):
        feats.append(tuple(range(f, f + 3)))
        f += 3
    assert f == F
    return feats


def _slots():
    """Production slots: each is ('q', fi) [2 kt tiles] or ('tp', fi, fj).

    Returns (slot list, kt tile count). Production order interleaves
    quads and triple-pairs so both exit lanes stay fed.
    """
    quads = list(range(NQ))
    tpairs = [(NQ + 2 * j, NQ + 2 * j + 1) for j in range(NT // 2)]
    slots = []
    qi, ti = 0, 0
    # interleave: roughly alternate to mix tile kinds
    while qi < len(quads) or ti < len(tpairs):
        if ti < len(tpairs):
            slots.append(('tp',) + tpairs[ti]); ti += 1
        if qi < len(quads):
            slots.append(('q', quads[qi])); qi += 1
    nkt = sum(2 if s[0] == 'q' else 1 for s in slots)
    return slots, nkt


def _build_nc():
    import concourse.bass as bass  # noqa: F401
    import concourse.mybir as mybir
    import concourse.tile as tile
    from concourse import bacc

    fp32 = mybir.dt.float32
    fp16 = mybir.dt.float16
    MUL = mybir.AluOpType.mult

    slots, nkt = _slots()
    nc = bacc.Bacc("TRN2", target_bir_lowering=False, debug=False)

    xt_d = nc.dram_tensor("xt", [nkt, 128, BLOC], fp16, kind="ExternalInput").ap()
    kr_d = nc.dram_tensor("kr", [nkt, 128, RULOC], fp16, kind="ExternalInput").ap()
    out_d = nc.dram_tensor("out", [NBT, 128, ULOC], fp32, kind="ExternalOutput").ap()

    with tile.TileContext(nc) as tc:
        with (
            tc.tile_pool(name="xt", bufs=6) as xpool,
            tc.tile_pool(name="kt", bufs=6) as kpool,
            tc.tile_pool(name="work", bufs=1) as wpool,
            tc.tile_pool(name="ps", bufs=4, space="PSUM") as pspool,
        ):

            # per-bt state (unique tags: same-tag tiles rotate/alias buffers)
            def wtile(shape, nm):
                return wpool.tile(shape, fp32, tag=nm, name=nm)

            def whtile(shape, nm):
                return wpool.tile(shape, fp16, tag=nm, name=nm)

            P = [wtile([128, RULOC], f"P{bt}") for bt in range(NBT)]
            SA = [wtile([128, RULOC], f"SA{bt}") for bt in range(NBT)]
            SQH = [[whtile([128, RULOC], f"SQ{bt}_{k}") for k in range(4)]
                   for bt in range(NBT)]
            PPH = [[whtile([128, RULOC], f"PP{bt}_{k}") for k in range(4)]
                   for bt in range(NBT)]
            FF = [wtile([128, RULOC], f"FF{bt}") for bt in range(NBT)]
            osum = [wtile([128, ULOC], f"os{bt}") for bt in range(NBT)]

            # per-bt consumption bookkeeping
            n_seen = [0] * NBT      # factor tiles consumed so far (route idx)
            n_dve = [0] * NBT       # DVE-chain tiles so far
            n_act = [0] * NBT       # Act-copied tiles so far
            n_pp = [0] * NBT        # fp16 pair-products made so far
            n_sa = [0] * NBT        # pair-products chained into SA so far

            def consume(bt, ps):
                """Route one produced factor tile (PSUM) for batch-tile bt."""
                i = n_seen[bt]; n_seen[bt] += 1
                if ROUTE[i] == 'v':
                    if n_dve[bt] == 0:
                        nc.vector.tensor_copy(P[bt][:], ps[:])
                    else:
                        nc.vector.tensor_mul(P[bt][:], P[bt][:], ps[:])
                    n_dve[bt] += 1
                    return
                k = n_act[bt]; n_act[bt] += 1
                q = SQH[bt][k % 4]
                nc.scalar.copy(q[:], ps[:])
                if k % 2 == 0:
                    return
                # fp16 pair-merge on DVE (2x_1p)
                j = n_pp[bt]; n_pp[bt] += 1
                pp = PPH[bt][j % 4]
                nc.vector.tensor_mul(pp[:], SQH[bt][(k - 1) % 4][:], q[:])
                # chain pair-products: first N_POOL_PP+1 into SA on Pool,
                # the last two fold into P on DVE at the end.
                if j == 0:
                    return                      # PPH[0] pending for SA init
                if j == 1:
                    nc.gpsimd.tensor_mul(SA[bt][:], PPH[bt][0][:], pp[:])
                    n_sa[bt] = 2
                elif j < N_POOL_PP + 1:
                    nc.gpsimd.tensor_mul(SA[bt][:], SA[bt][:], pp[:])
                    n_sa[bt] += 1
                else:
                    nc.vector.tensor_mul(P[bt][:], P[bt][:], pp[:])

            def load_tile(kti):
                kt = kpool.tile([128, RULOC], fp16, tag="kt")
                nc.sync.dma_start(kt[:], kr_d[kti])
                xt = xpool.tile([128, BLOC], fp16, tag="xt")
                nc.sync.dma_start(xt[:], xt_d[kti])
                return kt, xt

            kti = 0
            for s in slots:
                if s[0] == 'tp':
                    kt, xt = load_tile(kti)
                    kti += 1
                    for bt in range(NBT):
                        for band in range(2):
                            ps = pspool.tile([128, RULOC], fp32, tag="ps")
                            rs = slice(64 * band, 64 * band + 64)
                            for h in range(RULOC // 512):
                                hs = slice(512 * h, 512 * h + 512)
                                nc.tensor.matmul(
                                    ps[:, hs],
                                    xt[rs, 128 * bt : 128 * bt + 128],
                                    kt[rs, hs],
                                    start=True, stop=True,
                                    tile_position=(64 * band, 0),
                                )
                            consume(bt, ps)
                else:
                    kta, xta = load_tile(kti)
                    ktb, xtb = load_tile(kti + 1)
                    kti += 2
                    for bt in range(NBT):
                        ps = pspool.tile([128, RULOC], fp32, tag="ps")
                        for h in range(RULOC // 512):
                            hs = slice(512 * h, 512 * h + 512)
                            nc.tensor.matmul(
                                ps[:, hs],
                                xta[:, 128 * bt : 128 * bt + 128],
                                kta[:, hs],
                                start=True, stop=False,
                            )
                            nc.tensor.matmul(
                                ps[:, hs],
                                xtb[:, 128 * bt : 128 * bt + 128],
                                ktb[:, hs],
                                start=False, stop=True,
                            )
                        consume(bt, ps)
            assert kti == nkt

            for bt in range(NBT):
                assert n_seen[bt] == NFAC and n_pp[bt] == NPP
                nc.vector.tensor_mul(FF[bt][:], P[bt][:], SA[bt][:])
                nc.vector.tensor_reduce(
                    osum[bt][:],
                    FF[bt][:].rearrange("p (r u) -> p u r", r=R),
                    axis=mybir.AxisListType.X,
                    op=mybir.AluOpType.add,
                )
                nc.sync.dma_start(out_d[bt], osum[bt][:])

    nc.compile()
    return nc


def _outer_feats(A, feats):
    """Outer product of A[..., len(feats), D] slices over the feature axis.

    A: [N, F, D] -> returns [N, D^g] for the given feature tuple.
    """
    out = A[:, feats[0], :]
    for f in feats[1:]:
        out = (out[:, :, None] * A[:, f, :][:, None, :]).reshape(out.shape[0], -1)
    return out


def _host_prep(X, K):
    """Repack: per-core xt (batch-half) and kr (units-quarter), fp16.

    kt tile layout per slot:
      'tp' (fi, fj): rows 0-63 = K3[fi], rows 64-127 = K3[fj]; xt same bands.
      'q'  fi: two tiles, pass p rows = K4[fi][128p:128p+128]; xt same.
    """
    feats = _factor_feats()
    slots, nkt = _slots()

    # X-side: [B, D^g] per factor
    Xf = [_outer_feats(X, ft) for ft in feats]
    # K-side: [D^g, R*U] per factor
    Kt = K.transpose(2, 0, 1, 3)  # [F, D, R, U]
    Kf = []
    for ft in feats:
        out = Kt[ft[0]].reshape(D, R * U)
        for f in ft[1:]:
            out = (out[:, None, :] * Kt[f].reshape(1, D, R * U)).reshape(-1, R * U)
        Kf.append(out)

    # xt per batch-half: [nkt, 128, BLOC] fp16
    xts = []
    for cb in range(BB_SHARD):
        xt = np.zeros((nkt, 128, BLOC), dtype=np.float16)
        kti = 0
        bsl = slice(cb * BLOC, (cb + 1) * BLOC)
        for s in slots:
            if s[0] == 'tp':
                _, fi, fj = s
                xt[kti, 0:64] = Xf[fi][bsl].T
                xt[kti, 64:128] = Xf[fj][bsl].T
                kti += 1
            else:
                fi = s[1]
                x4 = Xf[fi][bsl]                    # [BLOC, 256]
                for p in range(2):
                    xt[kti] = x4[:, 128 * p : 128 * p + 128].T
                    kti += 1
        xts.append(xt)

    # kr per units-quarter: [nkt, 128, RULOC] fp16
    krs = []
    for cu in range(UB_SHARD):
        usl = np.arange(R)[:, None] * U + (cu * ULOC + np.arange(ULOC))[None, :]
        usl = usl.reshape(-1)                       # ru_loc = r*ULOC + u_loc
        kr = np.zeros((nkt, 128, RULOC), dtype=np.float16)
        kti = 0
        for s in slots:
            if s[0] == 'tp':
                _, fi, fj = s
                kr[kti, 0:64] = Kf[fi][:, usl]
                kr[kti, 64:128] = Kf[fj][:, usl]
                kti += 1
            else:
                k4 = Kf[s[1]][:, usl]               # [256, RULOC]
                for p in range(2):
                    kr[kti] = k4[128 * p : 128 * p + 128]
                    kti += 1
        krs.append(kr)
    return xts, krs


def _in_maps(xts, krs):
    return [
        {"xt": xts[c // UB_SHARD], "kr": krs[c % UB_SHARD]} for c in range(NCORES)
    ]


def kernel(**inputs):
    from concourse.bass_utils import run_bass_kernel_spmd

    X = np.asarray(inputs["X"], dtype=np.float32)
    K = np.asarray(inputs["kernel"], dtype=np.float32)
    assert X.shape == (B, F, D) and K.shape == (D, R, F, U)

    if "nc" not in _cached:
        _cached["nc"] = _build_nc()
    nc = _cached["nc"]

    xts, krs = _host_prep(X, K)
    res = run_bass_kernel_spmd(nc, _in_maps(xts, krs), core_ids=list(range(NCORES)))

    out = np.empty((B, U), dtype=np.float32)
    for c in range(NCORES):
        cb, cu = c // UB_SHARD, c % UB_SHARD
        blk = res.results[c]["out"].reshape(BLOC, ULOC)
        out[cb * BLOC : (cb + 1) * BLOC, cu * ULOC : (cu + 1) * ULOC] = blk
    return out


# revision 16
# speedup vs baseline: 1.2774x; 1.2774x over previous
"""Trainium2 Bass kernel for nn_CP_Based (CP-decomposition interaction layer).

Math (full problem):
    t[b,f,r,u] = sum_d X[b,f,d] * K[d,r,f,u]      (B=1024, F=64, D=4, R=32, U=128)
    had[b,r,u] = prod_f t[b,f,r,u]
    out[b,u]   = sum_r had[b,r,u]

Strategy (v2):
  * Shard (units x batch) = 4 x 2 over 8 cores: each core gets 512 batch
    rows (4 partition tiles) and 32 units (RU_loc = 32r*32u = 1024).
  * Feature grouping (host-side weight repack): merge g features into one
    "factor" via outer products of the per-feature slices; one K=4^g
    matmul computes the product of g per-feature terms. Mix of quads
    (g=4, K=256 -> 2 accumulating K=128 passes) and triples (g=3, K=64,
    two factors row-packed per PE pass via tile_position).
  * fp16 matmul inputs (fp32 PSUM accumulate): 1 cycle/row on PE (vs 4
    for fp32) and half the DMA bytes. Verified rel err ~8e-3 < 2e-2.
  * The hadamard over factors is elementwise and PSUM-exit-bound: PSUM
    can only be read by DVE and Act (1 elem/cycle/lane each; GpSimd and
    DMA have no PSUM port). Pipeline:
      - DVE chain: running product P *= ps directly from PSUM.
      - Act (ScalarE): copies other factor tiles PSUM->SBUF fp32.
      - Merges of copied tiles into a second running product SA via
        scalar_tensor_tensor (STT supports the 2x_2p DVE perf mode:
        0.5 cyc/elem fp32 all-SBUF), split between DVE and GpSimd.
      - Final: F = P*SA (STT), strided tensor_reduce over r, DMA out.

kernel() takes FULL inputs, repacks on host, runs SPMD on 8 cores,
reassembles the full [1024, 128] output.
"""

import numpy as np

B, F, D, R, U = 1024, 64, 4, 32, 128
NCORES = 8
UB_SHARD = 4                  # units split
BB_SHARD = 2                  # batch split
BLOC = B // BB_SHARD          # 512 batch rows per core
NBT = BLOC // 128             # 4 partition tiles per core
ULOC = U // UB_SHARD          # 32 units per core
RULOC = R * ULOC              # 1024

# Factor mix: NQ quads (4 feats) + NT triples (3 feats); 4*NQ + 3*NT = 64
NQ = 4
NT = 16
assert 4 * NQ + 3 * NT == F and NT % 2 == 0

# Per-factor-tile exit routing per batch-tile, in production order:
# 'v' -> DVE chain (P *= ps, fp32, ~1.4us/tile measured).
# 'a' -> Act copy PSUM->SBUF as fp16 (~1.25us; single t3 tiles are fp16
#        range-safe, unlike products). Pairs of copies merge on DVE via
#        fp16 tensor_tensor (2x_1p mode is real on HW: ~0.85us) into
#        fp16 pair-products PPH (range-safe to ~1e4 < 65504). The PPH
#        chain then runs in fp32 mostly on GpSimd (SBUF only, ~2.8us),
#        with the leftovers merged into P on DVE.
# fp32 gets no DVE perf mode on this silicon (STT 2x_2p is cost-model
# fiction), so balance is v*1.4 vs a*1.25 on Act + 0.85/pair on DVE.
NFAC = NQ + NT                # 20
# GpSimd is NOT used: trace analysis showed concurrent GpSimd tensor ops
# slow DVE ops 2-4x (shared SBUF path) — Pool "help" is negative-sum.
# v=2 DVE-direct tiles; 18 Act copies -> 9 fp16 pair merges -> fp32
# chain into P, all on DVE.
ROUTE = ['v', 'a', 'a', 'a', 'a', 'a', 'a', 'a', 'a', 'a', 'v', 'a',
         'a', 'a', 'a', 'a', 'a', 'a', 'a', 'a']
assert len(ROUTE) == NFAC and ROUTE.count('a') % 2 == 0
N_ACT = ROUTE.count('a')      # 18 -> 9 fp16 pair-products per bt
NPP = N_ACT // 2

_cached = {}


def _factor_feats():
    """Feature index tuples for each factor: NQ quads then NT triples."""
    feats = []
    f = 0
    for _ in range(NQ):
        feats.append(tuple(range(f, f + 4)))
        f += 4
    for _ in range(NT):
        feats.append(tuple(range(f, f + 3)))
        f += 3
    assert f == F
    return feats


def _slots():
    """Production slots: each is ('q', fi) [2 kt tiles] or ('tp', fi, fj).

    Returns (slot list, kt tile count). Production order interleaves
    quads and triple-pairs so both exit lanes stay fed.
    """
    quads = list(range(NQ))
    tpairs = [(NQ + 2 * j, NQ + 2 * j + 1) for j in range(NT // 2)]
    slots = []
    qi, ti = 0, 0
    # interleave: roughly alternate to mix tile kinds
    while qi < len(quads) or ti < len(tpairs):
        if ti < len(tpairs):
            slots.append(('tp',) + tpairs[ti]); ti += 1
        if qi < len(quads):
            slots.append(('q', quads[qi])); qi += 1
    nkt = sum(2 if s[0] == 'q' else 1 for s in slots)
    return slots, nkt


def _build_nc():
    import concourse.bass as bass  # noqa: F401
    import concourse.mybir as mybir
    import concourse.tile as tile
    from concourse import bacc

    fp32 = mybir.dt.float32
    fp16 = mybir.dt.float16
    MUL = mybir.AluOpType.mult

    slots, nkt = _slots()
    nc = bacc.Bacc("TRN2", target_bir_lowering=False, debug=False)

    xt_d = nc.dram_tensor("xt", [nkt, 128, BLOC], fp16, kind="ExternalInput").ap()
    kr_d = nc.dram_tensor("kr", [nkt, 128, RULOC], fp16, kind="ExternalInput").ap()
    out_d = nc.dram_tensor("out", [NBT, 128, ULOC], fp32, kind="ExternalOutput").ap()

    with tile.TileContext(nc) as tc:
        with (
            tc.tile_pool(name="xt", bufs=6) as xpool,
            tc.tile_pool(name="kt", bufs=6) as kpool,
            tc.tile_pool(name="work", bufs=1) as wpool,
            tc.tile_pool(name="ps", bufs=4, space="PSUM") as pspool,
        ):

            # per-bt state (unique tags: same-tag tiles rotate/alias buffers)
            def wtile(shape, nm):
                return wpool.tile(shape, fp32, tag=nm, name=nm)

            def whtile(shape, nm):
                return wpool.tile(shape, fp16, tag=nm, name=nm)

            P = [wtile([128, RULOC], f"P{bt}") for bt in range(NBT)]
            SQH = [[whtile([128, RULOC], f"SQ{bt}_{k}") for k in range(4)]
                   for bt in range(NBT)]
            PPH = [[whtile([128, RULOC], f"PP{bt}_{k}") for k in range(4)]
                   for bt in range(NBT)]
            osum = [wtile([128, ULOC], f"os{bt}") for bt in range(NBT)]

            # per-bt consumption bookkeeping
            n_seen = [0] * NBT      # factor tiles consumed so far (route idx)
            n_dve = [0] * NBT       # DVE-chain tiles so far
            n_act = [0] * NBT       # Act-copied tiles so far
            n_pp = [0] * NBT        # fp16 pair-products made so far

            def consume(bt, ps):
                """Route one produced factor tile (PSUM) for batch-tile bt."""
                i = n_seen[bt]; n_seen[bt] += 1
                if ROUTE[i] == 'v':
                    if n_dve[bt] == 0:
                        nc.vector.tensor_copy(P[bt][:], ps[:])
                    else:
                        nc.vector.tensor_mul(P[bt][:], P[bt][:], ps[:])
                    n_dve[bt] += 1
                    return
                k = n_act[bt]; n_act[bt] += 1
                q = SQH[bt][k % 4]
                nc.scalar.copy(q[:], ps[:])
                if k % 2 == 0:
                    return
                # fp16 pair-merge on DVE (2x_1p mode, ~830ns vs 1460 fp32)
                j = n_pp[bt]; n_pp[bt] += 1
                pp = PPH[bt][j % 4]
                nc.vector.tensor_mul(pp[:], SQH[bt][(k - 1) % 4][:], q[:])
                # fold pair-product into the running product (fp32, DVE)
                nc.vector.tensor_mul(P[bt][:], P[bt][:], pp[:])

            def load_tile(kti):
                kt = kpool.tile([128, RULOC], fp16, tag="kt")
                nc.sync.dma_start(kt[:], kr_d[kti])
                xt = xpool.tile([128, BLOC], fp16, tag="xt")
                nc.sync.dma_start(xt[:], xt_d[kti])
                return kt, xt

            kti = 0
            for s in slots:
                if s[0] == 'tp':
                    kt, xt = load_tile(kti)
                    kti += 1
                    for bt in range(NBT):
                        for band in range(2):
                            ps = pspool.tile([128, RULOC], fp32, tag="ps")
                            rs = slice(64 * band, 64 * band + 64)
                            for h in range(RULOC // 512):
                                hs = slice(512 * h, 512 * h + 512)
                                nc.tensor.matmul(
                                    ps[:, hs],
                                    xt[rs, 128 * bt : 128 * bt + 128],
                                    kt[rs, hs],
                                    start=True, stop=True,
                                    tile_position=(64 * band, 0),
                                )
                            consume(bt, ps)
                else:
                    kta, xta = load_tile(kti)
                    ktb, xtb = load_tile(kti + 1)
                    kti += 2
                    for bt in range(NBT):
                        ps = pspool.tile([128, RULOC], fp32, tag="ps")
                        for h in range(RULOC // 512):
                            hs = slice(512 * h, 512 * h + 512)
                            nc.tensor.matmul(
                                ps[:, hs],
                                xta[:, 128 * bt : 128 * bt + 128],
                                kta[:, hs],
                                start=True, stop=False,
                            )
                            nc.tensor.matmul(
                                ps[:, hs],
                                xtb[:, 128 * bt : 128 * bt + 128],
                                ktb[:, hs],
                                start=False, stop=True,
                            )
                        consume(bt, ps)
            assert kti == nkt

            for bt in range(NBT):
                assert n_seen[bt] == NFAC and n_pp[bt] == NPP
                nc.vector.tensor_reduce(
                    osum[bt][:],
                    P[bt][:].rearrange("p (r u) -> p u r", r=R),
                    axis=mybir.AxisListType.X,
                    op=mybir.AluOpType.add,
                )
                nc.sync.dma_start(out_d[bt], osum[bt][:])

    nc.compile()
    return nc


def _outer_feats(A, feats):
    """Outer product of A[..., len(feats), D] slices over the feature axis.

    A: [N, F, D] -> returns [N, D^g] for the given feature tuple.
    """
    out = A[:, feats[0], :]
    for f in feats[1:]:
        out = (out[:, :, None] * A[:, f, :][:, None, :]).reshape(out.shape[0], -1)
    return out


def _host_prep(X, K):
    """Repack: per-core xt (batch-half) and kr (units-quarter), fp16.

    kt tile layout per slot:
      'tp' (fi, fj): rows 0-63 = K3[fi], rows 64-127 = K3[fj]; xt same bands.
      'q'  fi: two tiles, pass p rows = K4[fi][128p:128p+128]; xt same.
    """
    feats = _factor_feats()
    slots, nkt = _slots()

    # X-side: [B, D^g] per factor
    Xf = [_outer_feats(X, ft) for ft in feats]
    # K-side: [D^g, R*U] per factor
    Kt = K.transpose(2, 0, 1, 3)  # [F, D, R, U]
    Kf = []
    for ft in feats:
        out = Kt[ft[0]].reshape(D, R * U)
        for f in ft[1:]:
            out = (out[:, None, :] * Kt[f].reshape(1, D, R * U)).reshape(-1, R * U)
        Kf.append(out)

    # xt per batch-half: [nkt, 128, BLOC] fp16
    xts = []
    for cb in range(BB_SHARD):
        xt = np.zeros((nkt, 128, BLOC), dtype=np.float16)
        kti = 0
        bsl = slice(cb * BLOC, (cb + 1) * BLOC)
        for s in slots:
            if s[0] == 'tp':
                _, fi, fj = s
                xt[kti, 0:64] = Xf[fi][bsl].T
                xt[kti, 64:128] = Xf[fj][bsl].T
                kti += 1
            else:
                fi = s[1]
                x4 = Xf[fi][bsl]                    # [BLOC, 256]
                for p in range(2):
                    xt[kti] = x4[:, 128 * p : 128 * p + 128].T
                    kti += 1
        xts.append(xt)

    # kr per units-quarter: [nkt, 128, RULOC] fp16
    krs = []
    for cu in range(UB_SHARD):
        usl = np.arange(R)[:, None] * U + (cu * ULOC + np.arange(ULOC))[None, :]
        usl = usl.reshape(-1)                       # ru_loc = r*ULOC + u_loc
        kr = np.zeros((nkt, 128, RULOC), dtype=np.float16)
        kti = 0
        for s in slots:
            if s[0] == 'tp':
                _, fi, fj = s
                kr[kti, 0:64] = Kf[fi][:, usl]
                kr[kti, 64:128] = Kf[fj][:, usl]
                kti += 1
            else:
                k4 = Kf[s[1]][:, usl]               # [256, RULOC]
                for p in range(2):
                    kr[kti] = k4[128 * p : 128 * p + 128]
                    kti += 1
        krs.append(kr)
    return xts, krs


def _in_maps(xts, krs):
    return [
        {"xt": xts[c // UB_SHARD], "kr": krs[c % UB_SHARD]} for c in range(NCORES)
    ]


def kernel(**inputs):
    from concourse.bass_utils import run_bass_kernel_spmd

    X = np.asarray(inputs["X"], dtype=np.float32)
    K = np.asarray(inputs["kernel"], dtype=np.float32)
    assert X.shape == (B, F, D) and K.shape == (D, R, F, U)

    if "nc" not in _cached:
        _cached["nc"] = _build_nc()
    nc = _cached["nc"]

    xts, krs = _host_prep(X, K)
    res = run_bass_kernel_spmd(nc, _in_maps(xts, krs), core_ids=list(range(NCORES)))

    out = np.empty((B, U), dtype=np.float32)
    for c in range(NCORES):
        cb, cu = c // UB_SHARD, c % UB_SHARD
        blk = res.results[c]["out"].reshape(BLOC, ULOC)
        out[cb * BLOC : (cb + 1) * BLOC, cu * ULOC : (cu + 1) * ULOC] = blk
    return out
